# revision 7
# baseline (speedup 1.0000x reference)
"""Trainium2 Bass kernel for nn_AdaptiveFourierFeatures.

Strategy
--------
The reference module computes, per batch b and token s:

    q[s,h,:]   depends on x[s] through two linear layers
    k[d,f,h,:] = f[d,f]*u[h,:] + v[h,:]         (keys are AFFINE in f[d,f]
                                                  because key_proj is Linear(1,A))
    scores[s,d,h,f] = q.k/sqrt(HD) = alpha[s,h]*f[d,f] + beta[s,h]

With the given inputs, freq_matrix*freq_scale has IDENTICAL rows
(f[d,:] == g[:] for all d), so softmax over f is d-independent and beta
cancels inside the softmax:

    attn[s,h,f] = softmax_f(alpha[s,h] * (g[f]-gc))      (gc: shift for range)
    aw[s,f]     = mean_h attn[s,h,f]

The fourier features contract with the MLP weights analytically using
sin(theta+phi) = sin*cos + cos*sin, folding phase and the D dimension into
small [F,O] matrices on the host.  The device pipeline per token is then:

    x(64) -> alpha-scores(64=H*F) -> softmax -> aw features z(32)
          -> [x|z|1](97) @ G(97x128) -> sigmoid*silu gate -> residual

Sharding: data-parallel over batch B=8, one batch element per NeuronCore.
All folded parameters are tiny and replicated.

kernel(**inputs) takes the FULL inputs and returns the FULL [B,S,D] output.
"""

import numpy as np
import ml_dtypes

# ---- problem constants (hardcoded; kernel.py must be self-contained) ----
B, S, D, F, A, H, O = 8, 2048, 64, 16, 32, 4, 64
HD = A // H
TWO_PI = 2.0 * np.pi
N_CORES = 8
HF = H * F            # 64 score columns per token
NFEAT = D + 2 * F + 1  # 97 = x | z_sin | z_cos | ones
HALF = S // 2          # stacked-half layout: 1024 tokens per half

BF16 = ml_dtypes.bfloat16

_CACHE = {}


def _build_program(nchunks: int = 2):
    """Build the 8-core SPMD bass program (per-core shapes)."""
    import concourse.bass as bass
    import concourse.bacc as bacc
    import concourse.tile as tile
    from concourse import mybir

    dt = mybir.dt
    AF = mybir.ActivationFunctionType
    ALU = mybir.AluOpType

    nc = bacc.Bacc("TRN2", target_bir_lowering=False, debug=False,
                   num_devices=N_CORES)

    # ---- per-core DRAM parameters ----
    xT = nc.dram_tensor("xT", [D, S], dt.bfloat16, kind="ExternalInput").ap()
    xn = nc.dram_tensor("xn", [S, D], dt.float32, kind="ExternalInput").ap()
    trig = nc.dram_tensor("trig", [2 * F, S], dt.bfloat16, kind="ExternalInput").ap()
    wsc = nc.dram_tensor("wsc", [D, HF], dt.bfloat16, kind="ExternalInput").ap()
    o1 = nc.dram_tensor("o1", [128, 8], dt.bfloat16, kind="ExternalInput").ap()
    e2q = nc.dram_tensor("e2q", [8, 128], dt.bfloat16, kind="ExternalInput").ap()
    o2 = nc.dram_tensor("o2", [128, 32], dt.bfloat16, kind="ExternalInput").ap()
    gmat = nc.dram_tensor("gmat", [NFEAT, 128], dt.bfloat16, kind="ExternalInput").ap()
    bsc = nc.dram_tensor("bsc", [128, 1], dt.float32, kind="ExternalInput").ap()
    out_d = nc.dram_tensor("out", [S, D], dt.float32, kind="ExternalOutput").ap()

    KT = S // 128                # 16 token tiles of 128
    CW = HALF // nchunks         # stacked-column chunk width

    with tile.TileContext(nc) as tc:
        with (
            tc.tile_pool(name="const", bufs=1) as cpool,
            tc.tile_pool(name="sb", bufs=1) as sb,
            tc.tile_pool(name="work", bufs=2) as wk,
            tc.tile_pool(name="ps", bufs=1, space="PSUM") as ps,
            tc.tile_pool(name="ps_big", bufs=1, space="PSUM") as psb,
        ):
            # ---- constants / inputs to SBUF ----
            c_wsc = cpool.tile([D, HF], dt.bfloat16)
            c_o1 = cpool.tile([128, 8], dt.bfloat16)
            c_e2q = cpool.tile([8, 128], dt.bfloat16)
            c_o2 = cpool.tile([128, 32], dt.bfloat16)
            c_g = cpool.tile([NFEAT, 128], dt.bfloat16)
            c_bsc = cpool.tile([128, 1], dt.float32)
            nc.sync.dma_start(out=c_wsc[:], in_=wsc[:])
            nc.sync.dma_start(out=c_o1[:], in_=o1[:])
            nc.sync.dma_start(out=c_e2q[:], in_=e2q[:])
            nc.sync.dma_start(out=c_o2[:], in_=o2[:])
            nc.sync.dma_start(out=c_g[:], in_=gmat[:])
            nc.sync.dma_start(out=c_bsc[:], in_=bsc[:])

            # trig table lives on partitions 64..95 to lane-align with CZ
            c_trig = cpool.tile([96, S], dt.bfloat16)
            nc.sync.dma_start(out=c_trig[64:96, :], in_=trig[:])

            # CZ = [x^T (0:64) | zs (64:80) | zc (80:96) | ones (96)]
            cz = sb.tile([NFEAT, S], dt.bfloat16)
            nc.sync.dma_start(out=cz[0:D, :], in_=xT[:])
            nc.vector.memset(cz[NFEAT - 1:NFEAT, :], 1.0)

            # natural-layout x for the residual: [128, (k,64)]
            xn_t = sb.tile([128, KT * D], dt.float32)
            nc.sync.dma_start(
                out=xn_t[:],
                in_=xn.rearrange("(k p) d -> p k d", p=128),
            )

            # warm up the activation table set (exp/tanh share one set)
            warm = cpool.tile([1, 2], dt.float32)
            nc.vector.memset(warm[:], 0.0)
            nc.scalar.activation(warm[:], warm[:], AF.Exp)

            out_t = sb.tile([128, KT * D], dt.float32)

            for c in range(nchunks):
                lo = c * CW                      # stacked column offset
                # token ranges covered by this chunk (one per half)
                tok_los = (lo, HALF + lo)

                # -- scores: S2[half*64+hf, col] = sum_d x^T[d, tok] Wsc[d, hf]
                s2 = ps.tile([128, CW], dt.float32, tag="s2")
                for h in range(2):
                    t0 = tok_los[h]
                    for n0 in range(0, CW, 512):
                        nn = min(512, CW - n0)
                        nc.tensor.matmul(
                            s2[h * 64:(h + 1) * 64, n0:n0 + nn],
                            c_wsc[:],
                            cz[0:D, t0 + n0:t0 + n0 + nn],
                            tile_position=(0, h * 64),
                        )

                # -- exp (bias adds the constant alpha-offset term)
                e1 = wk.tile([128, CW], dt.bfloat16, tag="e1")
                nc.scalar.activation(e1[:], s2[:], AF.Exp, bias=c_bsc[:])

                # -- denominators: den[(half,h), col] = sum_f e1
                den = ps.tile([8, CW], dt.float32, tag="den")
                for n0 in range(0, CW, 512):
                    nn = min(512, CW - n0)
                    nc.tensor.matmul(
                        den[:, n0:n0 + nn], c_o1[:], e1[:, n0:n0 + nn],
                        tile_position=(0, 0),
                    )

                # -- reciprocal + bf16 cast
                rec = wk.tile([8, CW], dt.float32, tag="rec")
                nc.vector.reciprocal(rec[:], den[:])
                recb = wk.tile([8, CW], dt.bfloat16, tag="recb")
                nc.vector.tensor_copy(recb[:], rec[:])

                # -- broadcast 1/den back to all 128 rows (x0.25 head-mean)
                rb = ps.tile([128, CW], dt.float32, tag="rb")
                for n0 in range(0, CW, 512):
                    nn = min(512, CW - n0)
                    nc.tensor.matmul(
                        rb[:, n0:n0 + nn], c_e2q[:], recb[:, n0:n0 + nn],
                        tile_position=(0, 0),
                    )

                # -- attn/4 = e1 * rb
                at = wk.tile([128, CW], dt.bfloat16, tag="at")
                nc.vector.tensor_mul(at[:], e1[:], rb[:])

                # -- aw rows (duplicated for sin/cos) on partitions 64..95
                aw = psb.tile([96, 2 * CW], dt.float32, tag="aw")
                for h in range(2):
                    for n0 in range(0, CW, 512):
                        nn = min(512, CW - n0)
                        nc.tensor.matmul(
                            aw[64:96, h * CW + n0:h * CW + n0 + nn],
                            c_o2[h * 64:(h + 1) * 64, :],
                            at[h * 64:(h + 1) * 64, n0:n0 + nn],
                            tile_position=(h * 64, 64),
                        )

                # -- z features into CZ rows 64..96 (aw * sin/cos table)
                for h in range(2):
                    t0 = tok_los[h]
                    nc.vector.tensor_mul(
                        cz[64:96, t0:t0 + CW],
                        aw[64:96, h * CW:(h + 1) * CW],
                        c_trig[64:96, t0:t0 + CW],
                    )

                # -- MLP: per 128-token tile, pre = CZ_tile^T @ G  (nat layout)
                pre = psb.tile([128, (KT // nchunks) * 128], dt.float32, tag="pre")
                kts = [t0 // 128 + i for t0 in tok_los for i in range(CW // 128)]
                for j, k in enumerate(kts):
                    nc.tensor.matmul(
                        pre[:, j * 128:(j + 1) * 128],
                        cz[:, k * 128:(k + 1) * 128],
                        c_g[:],
                        tile_position=(0, 0),
                    )

                # -- tanh(pre/2); sigmoid(a)=0.5+0.5*tanh(a/2)
                th = wk.tile([128, (KT // nchunks) * 128], dt.bfloat16, tag="th")
                nc.scalar.activation(th[:], pre[:], AF.Tanh, scale=0.5)

                nj = len(kts)
                kpc = CW // 128  # k-tiles per token range
                th_v = th[:].rearrange("p (j o) -> p j o", j=nj)
                pre_v = pre[:].rearrange("p (j o) -> p j o", j=nj)

                # -- w = (1+tanh_p) * pre_p   [silu*2]
                wt = wk.tile([128, nj * 64], dt.bfloat16, tag="wt")
                wt_v = wt[:].rearrange("p (j o) -> p j o", j=nj)
                nc.vector.scalar_tensor_tensor(
                    wt_v, th_v[:, :, 64:128], 1.0, pre_v[:, :, 64:128],
                    ALU.add, ALU.mult,
                )
                # -- gated*4 = (1+tanh_g) * w
                gt = wk.tile([128, nj * 64], dt.bfloat16, tag="gt")
                gt_v = gt[:].rearrange("p (j o) -> p j o", j=nj)
                nc.vector.scalar_tensor_tensor(
                    gt_v, th_v[:, :, 0:64], 1.0, wt_v, ALU.add, ALU.mult,
                )
                # -- out = gated*0.25 + x ; DMA out per token range
                out_v = out_d.rearrange("(k p) d -> p k d", p=128)
                for h in range(2):
                    k0 = tok_los[h] // 128
                    nc.vector.scalar_tensor_tensor(
                        out_t[:, k0 * 64:(k0 + kpc) * 64],
                        gt[:, h * kpc * 64:(h + 1) * kpc * 64], 0.25,
                        xn_t[:, k0 * 64:(k0 + kpc) * 64],
                        ALU.mult, ALU.add,
                    )
                    nc.sync.dma_start(
                        out=out_v[:, k0:k0 + kpc, :],
                        in_=out_t[:, k0 * 64:(k0 + kpc) * 64],
                    )

    nc.compile()
    return nc


def _fold_params(inputs):
    """Host-side constant folding (float64).  Returns per-core arrays."""
    f = (np.asarray(inputs["freq_matrix"], np.float64)
         * np.asarray(inputs["freq_scale"], np.float64))
    g = f[0]
    gc = 0.5 * (g.max() + g.min())
    gsh = g - gc

    Wq = np.asarray(inputs["Wq"], np.float64)
    bq = np.asarray(inputs["bq"], np.float64)
    Wk1 = np.asarray(inputs["Wk1"], np.float64)
    bk1 = np.asarray(inputs["bk1"], np.float64)
    Wqi = np.asarray(inputs["Wqi"], np.float64)
    bqi = np.asarray(inputs["bqi"], np.float64)
    Wki = np.asarray(inputs["Wki"], np.float64)
    bki = np.asarray(inputs["bki"], np.float64)
    ph = np.asarray(inputs["phase"], np.float64)

    u = Wki @ Wk1[:, 0]
    Wqq = Wqi @ Wq
    bqq = Wqi @ bq + bqi
    u_h = u.reshape(H, HD)
    M_alpha = np.einsum("he,hed->hd", u_h, Wqq.reshape(H, HD, D)) / np.sqrt(HD)
    c_alpha = np.einsum("he,he->h", u_h, bqq.reshape(H, HD)) / np.sqrt(HD)

    W_score = np.einsum("hd,f->dhf", M_alpha, gsh).reshape(D, HF)
    b_score = np.einsum("h,f->hf", c_alpha, gsh).reshape(HF)
    b_score2 = np.concatenate([b_score, b_score]).reshape(128, 1)

    t = np.linspace(0.0, 1.0, S)
    theta = TWO_PI * t[:, None] * g[None, :]
    trig = np.concatenate([np.sin(theta).T, np.cos(theta).T], 0)  # [2F, S]

    cph, sph = np.cos(ph), np.sin(ph)

    def fold_mlp(W):
        W = np.asarray(W, np.float64)
        Wx = W[:, :D]
        Wf = W[:, D:].reshape(O, D, 2 * F)
        Ws, Wc = Wf[:, :, :F], Wf[:, :, F:]
        Us = np.einsum("df,odf->fo", cph, Ws) - np.einsum("df,odf->fo", sph, Wc)
        Uc = np.einsum("df,odf->fo", sph, Ws) + np.einsum("df,odf->fo", cph, Wc)
        return Wx, Us, Uc

    Wgx, Ugs, Ugc = fold_mlp(inputs["Wg"])
    Wpx, Ups, Upc = fold_mlp(inputs["Wp"])
    bg = np.asarray(inputs["bg"], np.float64)
    bp = np.asarray(inputs["bp"], np.float64)

    G = np.zeros((NFEAT, 128))
    G[0:D, 0:64] = Wgx.T
    G[D:D + F, 0:64] = Ugs
    G[D + F:D + 2 * F, 0:64] = Ugc
    G[NFEAT - 1, 0:64] = bg
    G[0:D, 64:128] = Wpx.T
    G[D:D + F, 64:128] = Ups
    G[D + F:D + 2 * F, 64:128] = Upc
    G[NFEAT - 1, 64:128] = bp

    # indicator matrices for the softmax plumbing
    p = np.arange(128)
    O1 = (p[:, None] // 16 == np.arange(8)[None, :]).astype(np.float64)
    E2q = 0.25 * (np.arange(8)[:, None] == p[None, :] // 16).astype(np.float64)
    O2 = ((p[:, None] % 16) == (np.arange(32)[None, :] % 16)).astype(np.float64)

    return dict(
        trig=trig.astype(BF16),
        wsc=W_score.astype(BF16),
        o1=O1.astype(BF16),
        e2q=E2q.astype(BF16),
        o2=O2.astype(BF16),
        gmat=G.astype(BF16),
        bsc=b_score2.astype(np.float32),
    ), gsh, M_alpha, c_alpha


def _numpy_fallback(inputs):
    """Exact collapsed computation in numpy (general freq rows)."""
    x = np.asarray(inputs["x"], np.float64)
    f = (np.asarray(inputs["freq_matrix"], np.float64)
         * np.asarray(inputs["freq_scale"], np.float64))
    Wq = np.asarray(inputs["Wq"], np.float64); bq = np.asarray(inputs["bq"], np.float64)
    Wk1 = np.asarray(inputs["Wk1"], np.float64); bk1 = np.asarray(inputs["bk1"], np.float64)
    Wqi = np.asarray(inputs["Wqi"], np.float64); bqi = np.asarray(inputs["bqi"], np.float64)
    Wki = np.asarray(inputs["Wki"], np.float64); bki = np.asarray(inputs["bki"], np.float64)
    Wg = np.asarray(inputs["Wg"], np.float64); bg = np.asarray(inputs["bg"], np.float64)
    Wp = np.asarray(inputs["Wp"], np.float64); bp = np.asarray(inputs["bp"], np.float64)
    ph = np.asarray(inputs["phase"], np.float64)

    u = Wki @ Wk1[:, 0]
    v = Wki @ bk1 + bki
    q = (x @ Wq.T + bq) @ Wqi.T + bqi                      # [B,S,A]
    qh = q.reshape(B, S, H, HD)
    alpha = np.einsum("bshe,he->bsh", qh, u.reshape(H, HD)) / np.sqrt(HD)
    beta = np.einsum("bshe,he->bsh", qh, v.reshape(H, HD)) / np.sqrt(HD)
    sc = alpha[..., None, :, None] * f[None, None, :, None, :] \
        + beta[..., None, :, None]                         # [B,S,D,H,F]
    sc -= sc.max(-1, keepdims=True)
    e = np.exp(sc)
    attn = e / e.sum(-1, keepdims=True)
    aw = attn.mean(-2)                                     # [B,S,D,F]
    t = np.linspace(0.0, 1.0, S)
    sig = TWO_PI * t[None, :, None, None] * f[None, None] + ph[None, None]
    ffs = np.sin(sig) * aw
    ffc = np.cos(sig) * aw
    ff = np.concatenate([ffs, ffc], axis=-1).reshape(B, S, D * 2 * F)
    ci = np.concatenate([x, ff], axis=-1)
    gate = 1.0 / (1.0 + np.exp(-(ci @ Wg.T + bg)))
    pp = ci @ Wp.T + bp
    silu = pp / (1.0 + np.exp(-pp))
    return (x + gate * silu).astype(np.float32)


def kernel(**inputs) -> np.ndarray:
    x = np.asarray(inputs["x"], np.float32)

    f = (np.asarray(inputs["freq_matrix"], np.float64)
         * np.asarray(inputs["freq_scale"], np.float64))
    if not np.all(f == f[0:1]):
        return _numpy_fallback(inputs)

    params, gsh, M_alpha, c_alpha = _fold_params(inputs)

    # exp-overflow guard (score = alpha*(g-gc); needs |score| < ~85)
    xmaxn = np.linalg.norm(x.reshape(-1, D), axis=1).max()
    amax = np.linalg.norm(M_alpha, axis=1).max() * xmaxn + np.abs(c_alpha).max()
    if amax * np.abs(gsh).max() > 85.0:
        return _numpy_fallback(inputs)

    key = "prog"
    if key not in _CACHE:
        _CACHE[key] = _build_program()
    nc = _CACHE[key]

    from concourse.bass_utils import run_bass_kernel_spmd

    in_maps = []
    for b in range(N_CORES):
        m = dict(params)
        m["xT"] = np.ascontiguousarray(x[b].T).astype(BF16)
        m["xn"] = np.ascontiguousarray(x[b])
        in_maps.append(m)

    res = run_bass_kernel_spmd(nc, in_maps, core_ids=list(range(N_CORES)))
    out = np.stack([res.results[b]["out"] for b in range(N_CORES)], axis=0)
    return out.astype(np.float32)


if __name__ == "__main__":
    import reference
    ins = {k: np.asarray(v) for k, v in reference.setup_inputs().items()}
    got = kernel(**ins)
    import jax.numpy as jnp
    exp = np.asarray(reference.reference(**{k: jnp.asarray(v) for k, v in ins.items()}))
    err = np.linalg.norm(got - exp) / np.linalg.norm(exp)
    print("rel err:", err)


# revision 18
# speedup vs baseline: 1.0660x; 1.0660x over previous
"""Trainium2 Bass kernel for nn_AdaptiveFourierFeatures.

Strategy
--------
The reference module computes, per batch b and token s:

    q[s,h,:]   depends on x[s] through two linear layers
    k[d,f,h,:] = f[d,f]*u[h,:] + v[h,:]         (keys are AFFINE in f[d,f]
                                                  because key_proj is Linear(1,A))
    scores[s,d,h,f] = q.k/sqrt(HD) = alpha[s,h]*f[d,f] + beta[s,h]

With the given inputs, freq_matrix*freq_scale has IDENTICAL rows
(f[d,:] == g[:] for all d), so softmax over f is d-independent and beta
cancels inside the softmax:

    attn[s,h,f] = softmax_f(alpha[s,h] * (g[f]-gc))      (gc: shift for range)
    aw[s,f]     = mean_h attn[s,h,f]

The fourier features contract with the MLP weights analytically using
sin(theta+phi) = sin*cos + cos*sin, folding phase and the D dimension into
small [F,O] matrices on the host.  The device pipeline per token is then:

    x(64) -> alpha-scores(64=H*F) -> softmax -> aw features z(32)
          -> [x|z|1](97) @ G(97x128) -> sigmoid*silu gate -> residual

Sharding: data-parallel over batch B=8, one batch element per NeuronCore.
All folded parameters are tiny and replicated.

kernel(**inputs) takes the FULL inputs and returns the FULL [B,S,D] output.
"""

import numpy as np
import ml_dtypes

# ---- problem constants (hardcoded; kernel.py must be self-contained) ----
B, S, D, F, A, H, O = 8, 2048, 64, 16, 32, 4, 64
HD = A // H
TWO_PI = 2.0 * np.pi
N_CORES = 8
HF = H * F            # 64 score columns per token
NFEAT = D + 2 * F + 1  # 97 = x | z_sin | z_cos | ones
HALF = S // 2          # stacked-half layout: 1024 tokens per half

BF16 = ml_dtypes.bfloat16

_CACHE = {}


def _build_program(nchunks: int = 2):
    """Build the 8-core SPMD bass program (per-core shapes)."""
    import concourse.bass as bass
    import concourse.bacc as bacc
    import concourse.tile as tile
    from concourse import mybir

    dt = mybir.dt
    AF = mybir.ActivationFunctionType
    ALU = mybir.AluOpType

    nc = bacc.Bacc("TRN2", target_bir_lowering=False, debug=False,
                   num_devices=N_CORES)

    # ---- per-core DRAM parameters ----
    # all bf16 params packed into one [128, 360] array:
    #   wsc [64,64] @cols 0:64, o1 [128,8] @64:72, e2q [8,128] @72:200,
    #   o2 [128,32] @200:232, G [97,128] @232:360
    xT = nc.dram_tensor("xT", [D, S], dt.bfloat16, kind="ExternalInput").ap()
    xn = nc.dram_tensor("xn", [S, D], dt.float32, kind="ExternalInput").ap()
    trig = nc.dram_tensor("trig", [2 * F, S], dt.bfloat16, kind="ExternalInput").ap()
    pk = nc.dram_tensor("pk", [128, 360], dt.bfloat16, kind="ExternalInput").ap()
    bsc = nc.dram_tensor("bsc", [128, 1], dt.float32, kind="ExternalInput").ap()
    out_d = nc.dram_tensor("out", [S, D], dt.float32, kind="ExternalOutput").ap()

    KT = S // 128                # 16 token tiles of 128
    CW = HALF // nchunks         # stacked-column chunk width

    with tile.TileContext(nc) as tc:
        with (
            tc.tile_pool(name="const", bufs=1) as cpool,
            tc.tile_pool(name="sb", bufs=1) as sb,
            tc.tile_pool(name="work", bufs=2) as wk,
            tc.tile_pool(name="ps2", bufs=2, space="PSUM") as ps2,
            tc.tile_pool(name="ps", bufs=1, space="PSUM") as ps,
            tc.tile_pool(name="ps_big", bufs=1, space="PSUM") as psb,
        ):
            # ---- inputs to SBUF (x first — it gates compute; two HWDGE rings) ----
            # CZ = [x^T (0:64) | zs (64:80) | zc (80:96) | ones (96)]
            cz = sb.tile([NFEAT, S], dt.bfloat16)
            nc.sync.dma_start(out=cz[0:D, :], in_=xT[:])

            # natural-layout x for the residual: [128, (k,64)]
            xn_t = sb.tile([128, KT * D], dt.float32)
            nc.scalar.dma_start(
                out=xn_t[:],
                in_=xn.rearrange("(k p) d -> p k d", p=128),
            )

            c_pk = cpool.tile([128, 360], dt.bfloat16)
            nc.sync.dma_start(out=c_pk[:], in_=pk[:])
            c_wsc = c_pk[0:D, 0:64]
            c_o1 = c_pk[0:128, 64:72]
            c_e2q = c_pk[0:8, 72:200]
            c_o2 = c_pk[0:128, 200:232]
            c_g = c_pk[0:NFEAT, 232:360]

            c_bsc = cpool.tile([128, 1], dt.float32)
            nc.scalar.dma_start(out=c_bsc[:], in_=bsc[:])

            # trig table lives on partitions 64..95 to lane-align with CZ
            c_trig = cpool.tile([96, S], dt.bfloat16)
            nc.sync.dma_start(out=c_trig[64:96, :], in_=trig[:])

            nc.vector.memset(cz[NFEAT - 1:NFEAT, :], 1.0)

            # warm up the activation table set (exp/tanh share one set)
            warm = cpool.tile([1, 2], dt.float32)
            nc.vector.memset(warm[:], 0.0)
            nc.scalar.activation(warm[:], warm[:], AF.Exp)

            out_t = sb.tile([128, KT * D], dt.float32)

            for c in range(nchunks):
                lo = c * CW                      # stacked column offset
                # token ranges covered by this chunk (one per half)
                tok_los = (lo, HALF + lo)

                # -- scores: S2[half*64+hf, col] = sum_d x^T[d, tok] Wsc[d, hf]
                s2 = ps2.tile([128, CW], dt.float32, tag="s2")
                for h in range(2):
                    t0 = tok_los[h]
                    for n0 in range(0, CW, 512):
                        nn = min(512, CW - n0)
                        nc.tensor.matmul(
                            s2[h * 64:(h + 1) * 64, n0:n0 + nn],
                            c_wsc,
                            cz[0:D, t0 + n0:t0 + n0 + nn],
                            tile_position=(0, h * 64),
                        )

                # -- exp (bias adds the constant alpha-offset term)
                e1 = wk.tile([128, CW], dt.bfloat16, tag="e1")
                nc.scalar.activation(e1[:], s2[:], AF.Exp, bias=c_bsc[:])

                # -- denominators: den[(half,h), col] = sum_f e1
                den = ps.tile([8, CW], dt.float32, tag="den")
                for n0 in range(0, CW, 512):
                    nn = min(512, CW - n0)
                    nc.tensor.matmul(
                        den[:, n0:n0 + nn], c_o1, e1[:, n0:n0 + nn],
                        tile_position=(0, 0),
                    )

                # -- reciprocal (fast Newton approx, ~18 bits) + bf16 cast
                rec = wk.tile([8, CW], dt.float32, tag="rec")
                nc.vector.reciprocal_approx_fast(rec[:], den[:])
                recb = wk.tile([8, CW], dt.bfloat16, tag="recb")
                nc.vector.tensor_copy(recb[:], rec[:])

                # -- broadcast 1/den back to all 128 rows (x0.25 head-mean)
                rb = ps.tile([128, CW], dt.float32, tag="rb")
                for n0 in range(0, CW, 512):
                    nn = min(512, CW - n0)
                    nc.tensor.matmul(
                        rb[:, n0:n0 + nn], c_e2q, recb[:, n0:n0 + nn],
                        tile_position=(0, 0),
                    )

                # -- attn/4 = e1 * rb
                at = wk.tile([128, CW], dt.bfloat16, tag="at")
                nc.vector.tensor_mul(at[:], e1[:], rb[:])

                # -- aw rows (duplicated for sin/cos) on partitions 64..95
                aw = psb.tile([96, 2 * CW], dt.float32, tag="aw")
                for h in range(2):
                    for n0 in range(0, CW, 512):
                        nn = min(512, CW - n0)
                        nc.tensor.matmul(
                            aw[64:96, h * CW + n0:h * CW + n0 + nn],
                            c_o2[h * 64:(h + 1) * 64, :],
                            at[h * 64:(h + 1) * 64, n0:n0 + nn],
                            tile_position=(h * 64, 64),
                        )

                # -- z features into CZ rows 64..96 (aw * sin/cos table)
                for h in range(2):
                    t0 = tok_los[h]
                    nc.vector.tensor_mul(
                        cz[64:96, t0:t0 + CW],
                        aw[64:96, h * CW:(h + 1) * CW],
                        c_trig[64:96, t0:t0 + CW],
                    )

                # -- MLP: per 128-token tile, pre = CZ_tile^T @ G  (nat layout)
                pre = psb.tile([128, (KT // nchunks) * 128], dt.float32, tag="pre")
                kts = [t0 // 128 + i for t0 in tok_los for i in range(CW // 128)]
                for j, k in enumerate(kts):
                    nc.tensor.matmul(
                        pre[:, j * 128:(j + 1) * 128],
                        cz[:, k * 128:(k + 1) * 128],
                        c_g,
                        tile_position=(0, 0),
                    )

                # -- tanh(pre/2); sigmoid(a)=0.5+0.5*tanh(a/2)
                th = wk.tile([128, (KT // nchunks) * 128], dt.bfloat16, tag="th")
                nc.scalar.activation(th[:], pre[:], AF.Tanh, scale=0.5)

                nj = len(kts)
                kpc = CW // 128  # k-tiles per token range
                th_v = th[:].rearrange("p (j o) -> p j o", j=nj)
                pre_v = pre[:].rearrange("p (j o) -> p j o", j=nj)

                # -- w = (1+tanh_p) * pre_p   [silu*2]
                wt = wk.tile([128, nj * 64], dt.bfloat16, tag="wt")
                wt_v = wt[:].rearrange("p (j o) -> p j o", j=nj)
                nc.vector.scalar_tensor_tensor(
                    wt_v, th_v[:, :, 64:128], 1.0, pre_v[:, :, 64:128],
                    ALU.add, ALU.mult,
                )
                # -- gated*4 = (1+tanh_g) * w
                gt = wk.tile([128, nj * 64], dt.bfloat16, tag="gt")
                gt_v = gt[:].rearrange("p (j o) -> p j o", j=nj)
                nc.vector.scalar_tensor_tensor(
                    gt_v, th_v[:, :, 0:64], 1.0, wt_v, ALU.add, ALU.mult,
                )
                # -- out = gated*0.25 + x ; DMA out per token range
                out_v = out_d.rearrange("(k p) d -> p k d", p=128)
                for h in range(2):
                    k0 = tok_los[h] // 128
                    nc.vector.scalar_tensor_tensor(
                        out_t[:, k0 * 64:(k0 + kpc) * 64],
                        gt[:, h * kpc * 64:(h + 1) * kpc * 64], 0.25,
                        xn_t[:, k0 * 64:(k0 + kpc) * 64],
                        ALU.mult, ALU.add,
                    )
                    eng = nc.sync if h == 0 else nc.scalar
                    eng.dma_start(
                        out=out_v[:, k0:k0 + kpc, :],
                        in_=out_t[:, k0 * 64:(k0 + kpc) * 64],
                    )

    nc.compile()
    return nc


def _fold_params(inputs):
    """Host-side constant folding (float64).  Returns per-core arrays."""
    f = (np.asarray(inputs["freq_matrix"], np.float64)
         * np.asarray(inputs["freq_scale"], np.float64))
    g = f[0]
    gc = 0.5 * (g.max() + g.min())
    gsh = g - gc

    Wq = np.asarray(inputs["Wq"], np.float64)
    bq = np.asarray(inputs["bq"], np.float64)
    Wk1 = np.asarray(inputs["Wk1"], np.float64)
    bk1 = np.asarray(inputs["bk1"], np.float64)
    Wqi = np.asarray(inputs["Wqi"], np.float64)
    bqi = np.asarray(inputs["bqi"], np.float64)
    Wki = np.asarray(inputs["Wki"], np.float64)
    bki = np.asarray(inputs["bki"], np.float64)
    ph = np.asarray(inputs["phase"], np.float64)

    u = Wki @ Wk1[:, 0]
    Wqq = Wqi @ Wq
    bqq = Wqi @ bq + bqi
    u_h = u.reshape(H, HD)
    M_alpha = np.einsum("he,hed->hd", u_h, Wqq.reshape(H, HD, D)) / np.sqrt(HD)
    c_alpha = np.einsum("he,he->h", u_h, bqq.reshape(H, HD)) / np.sqrt(HD)

    W_score = np.einsum("hd,f->dhf", M_alpha, gsh).reshape(D, HF)
    b_score = np.einsum("h,f->hf", c_alpha, gsh).reshape(HF)
    b_score2 = np.concatenate([b_score, b_score]).reshape(128, 1)

    t = np.linspace(0.0, 1.0, S)
    theta = TWO_PI * t[:, None] * g[None, :]
    trig = np.concatenate([np.sin(theta).T, np.cos(theta).T], 0)  # [2F, S]

    cph, sph = np.cos(ph), np.sin(ph)

    def fold_mlp(W):
        W = np.asarray(W, np.float64)
        Wx = W[:, :D]
        Wf = W[:, D:].reshape(O, D, 2 * F)
        Ws, Wc = Wf[:, :, :F], Wf[:, :, F:]
        Us = np.einsum("df,odf->fo", cph, Ws) - np.einsum("df,odf->fo", sph, Wc)
        Uc = np.einsum("df,odf->fo", sph, Ws) + np.einsum("df,odf->fo", cph, Wc)
        return Wx, Us, Uc

    Wgx, Ugs, Ugc = fold_mlp(inputs["Wg"])
    Wpx, Ups, Upc = fold_mlp(inputs["Wp"])
    bg = np.asarray(inputs["bg"], np.float64)
    bp = np.asarray(inputs["bp"], np.float64)

    G = np.zeros((NFEAT, 128))
    G[0:D, 0:64] = Wgx.T
    G[D:D + F, 0:64] = Ugs
    G[D + F:D + 2 * F, 0:64] = Ugc
    G[NFEAT - 1, 0:64] = bg
    G[0:D, 64:128] = Wpx.T
    G[D:D + F, 64:128] = Ups
    G[D + F:D + 2 * F, 64:128] = Upc
    G[NFEAT - 1, 64:128] = bp

    # indicator matrices for the softmax plumbing
    p = np.arange(128)
    O1 = (p[:, None] // 16 == np.arange(8)[None, :]).astype(np.float64)
    E2q = 0.25 * (np.arange(8)[:, None] == p[None, :] // 16).astype(np.float64)
    O2 = ((p[:, None] % 16) == (np.arange(32)[None, :] % 16)).astype(np.float64)

    # pack all bf16 params into one [128, 360] array (see _build_program)
    pk = np.zeros((128, 360))
    pk[0:D, 0:64] = W_score
    pk[0:128, 64:72] = O1
    pk[0:8, 72:200] = E2q
    pk[0:128, 200:232] = O2
    pk[0:NFEAT, 232:360] = G

    return dict(
        trig=trig.astype(BF16),
        pk=pk.astype(BF16),
        bsc=b_score2.astype(np.float32),
    ), gsh, M_alpha, c_alpha


def _numpy_fallback(inputs):
    """Exact collapsed computation in numpy (general freq rows)."""
    x = np.asarray(inputs["x"], np.float64)
    f = (np.asarray(inputs["freq_matrix"], np.float64)
         * np.asarray(inputs["freq_scale"], np.float64))
    Wq = np.asarray(inputs["Wq"], np.float64); bq = np.asarray(inputs["bq"], np.float64)
    Wk1 = np.asarray(inputs["Wk1"], np.float64); bk1 = np.asarray(inputs["bk1"], np.float64)
    Wqi = np.asarray(inputs["Wqi"], np.float64); bqi = np.asarray(inputs["bqi"], np.float64)
    Wki = np.asarray(inputs["Wki"], np.float64); bki = np.asarray(inputs["bki"], np.float64)
    Wg = np.asarray(inputs["Wg"], np.float64); bg = np.asarray(inputs["bg"], np.float64)
    Wp = np.asarray(inputs["Wp"], np.float64); bp = np.asarray(inputs["bp"], np.float64)
    ph = np.asarray(inputs["phase"], np.float64)

    u = Wki @ Wk1[:, 0]
    v = Wki @ bk1 + bki
    q = (x @ Wq.T + bq) @ Wqi.T + bqi                      # [B,S,A]
    qh = q.reshape(B, S, H, HD)
    alpha = np.einsum("bshe,he->bsh", qh, u.reshape(H, HD)) / np.sqrt(HD)
    beta = np.einsum("bshe,he->bsh", qh, v.reshape(H, HD)) / np.sqrt(HD)
    sc = alpha[..., None, :, None] * f[None, None, :, None, :] \
        + beta[..., None, :, None]                         # [B,S,D,H,F]
    sc -= sc.max(-1, keepdims=True)
    e = np.exp(sc)
    attn = e / e.sum(-1, keepdims=True)
    aw = attn.mean(-2)                                     # [B,S,D,F]
    t = np.linspace(0.0, 1.0, S)
    sig = TWO_PI * t[None, :, None, None] * f[None, None] + ph[None, None]
    ffs = np.sin(sig) * aw
    ffc = np.cos(sig) * aw
    ff = np.concatenate([ffs, ffc], axis=-1).reshape(B, S, D * 2 * F)
    ci = np.concatenate([x, ff], axis=-1)
    gate = 1.0 / (1.0 + np.exp(-(ci @ Wg.T + bg)))
    pp = ci @ Wp.T + bp
    silu = pp / (1.0 + np.exp(-pp))
    return (x + gate * silu).astype(np.float32)


def kernel(**inputs) -> np.ndarray:
    x = np.asarray(inputs["x"], np.float32)

    f = (np.asarray(inputs["freq_matrix"], np.float64)
         * np.asarray(inputs["freq_scale"], np.float64))
    if not np.all(f == f[0:1]):
        return _numpy_fallback(inputs)

    params, gsh, M_alpha, c_alpha = _fold_params(inputs)

    # exp-overflow guard (score = alpha*(g-gc); needs |score| < ~85)
    xmaxn = np.linalg.norm(x.reshape(-1, D), axis=1).max()
    amax = np.linalg.norm(M_alpha, axis=1).max() * xmaxn + np.abs(c_alpha).max()
    if amax * np.abs(gsh).max() > 85.0:
        return _numpy_fallback(inputs)

    key = "prog"
    if key not in _CACHE:
        _CACHE[key] = _build_program()
    nc = _CACHE[key]

    from concourse.bass_utils import run_bass_kernel_spmd

    in_maps = []
    for b in range(N_CORES):
        m = dict(params)
        m["xT"] = np.ascontiguousarray(x[b].T).astype(BF16)
        m["xn"] = np.ascontiguousarray(x[b])
        in_maps.append(m)

    res = run_bass_kernel_spmd(nc, in_maps, core_ids=list(range(N_CORES)))
    out = np.stack([res.results[b]["out"] for b in range(N_CORES)], axis=0)
    return out.astype(np.float32)


if __name__ == "__main__":
    import reference
    ins = {k: np.asarray(v) for k, v in reference.setup_inputs().items()}
    got = kernel(**ins)
    import jax.numpy as jnp
    exp = np.asarray(reference.reference(**{k: jnp.asarray(v) for k, v in ins.items()}))
    err = np.linalg.norm(got - exp) / np.linalg.norm(exp)
    print("rel err:", err)


# revision 25
# speedup vs baseline: 1.2218x; 1.1461x over previous
"""Trainium2 Bass kernel for nn_AdaptiveFourierFeatures.

Strategy
--------
The reference module computes, per batch b and token s:

    q[s,h,:]   depends on x[s] through two linear layers
    k[d,f,h,:] = f[d,f]*u[h,:] + v[h,:]         (keys are AFFINE in f[d,f]
                                                  because key_proj is Linear(1,A))
    scores[s,d,h,f] = q.k/sqrt(HD) = alpha[s,h]*f[d,f] + beta[s,h]

With the given inputs, freq_matrix*freq_scale has IDENTICAL rows
(f[d,:] == g[:] for all d), so softmax over f is d-independent and beta
cancels inside the softmax:

    attn[s,h,f] = softmax_f(alpha[s,h] * (g[f]-gc))      (gc: shift for range)
    aw[s,f]     = mean_h attn[s,h,f]

The fourier features contract with the MLP weights analytically using
sin(theta+phi) = sin*cos + cos*sin, folding phase and the D dimension into
small [F,O] matrices on the host.  The device pipeline per token is then:

    x(64) -> alpha-scores(64=H*F) -> softmax -> aw features z(32)
          -> [x|z|1](97) @ G(97x128) -> sigmoid*silu gate -> residual

Sharding: data-parallel over batch B=8, one batch element per NeuronCore.
All folded parameters are tiny and replicated.

kernel(**inputs) takes the FULL inputs and returns the FULL [B,S,D] output.
"""

import numpy as np
import ml_dtypes

# ---- problem constants (hardcoded; kernel.py must be self-contained) ----
B, S, D, F, A, H, O = 8, 2048, 64, 16, 32, 4, 64
HD = A // H
TWO_PI = 2.0 * np.pi
N_CORES = 8
HF = H * F            # 64 score columns per token
NFEAT = D + 2 * F + 1  # 97 = x | z_sin | z_cos | ones
HALF = S // 2          # stacked-half layout: 1024 tokens per half

BF16 = ml_dtypes.bfloat16

_CACHE = {}


def _build_program(nchunks: int = 2):
    """Build the 8-core SPMD bass program (per-core shapes)."""
    import concourse.bass as bass
    import concourse.bacc as bacc
    import concourse.tile as tile
    from concourse import mybir
    from bass_rust import add_dep_helper

    dt = mybir.dt
    AF = mybir.ActivationFunctionType
    ALU = mybir.AluOpType

    nc = bacc.Bacc("TRN2", target_bir_lowering=False, debug=False,
                   num_devices=N_CORES)

    # ---- per-core DRAM parameters ----
    # all bf16 params packed into one [128, 361] array:
    #   wsc [64,64] @cols 0:64, o1 [128,8] @64:72, e2q [8,128] @72:200,
    #   o2 [128,32] @200:232, G [97,128] @232:360, b_score [128,1] @360
    xT = nc.dram_tensor("xT", [D, S], dt.bfloat16, kind="ExternalInput").ap()
    xn = nc.dram_tensor("xn", [S, D], dt.float32, kind="ExternalInput").ap()
    trig = nc.dram_tensor("trig", [2 * F, S], dt.bfloat16, kind="ExternalInput").ap()
    pk = nc.dram_tensor("pk", [128, 361], dt.bfloat16, kind="ExternalInput").ap()
    out_d = nc.dram_tensor("out", [S, D], dt.float32, kind="ExternalOutput").ap()

    KT = S // 128                # 16 token tiles of 128
    CW = HALF // nchunks         # stacked-column chunk width

    with tile.TileContext(nc) as tc:
        with (
            tc.tile_pool(name="const", bufs=1) as cpool,
            tc.tile_pool(name="sb", bufs=1) as sb,
            tc.tile_pool(name="work", bufs=2) as wk,
            tc.tile_pool(name="ps2", bufs=2, space="PSUM") as ps2,
            tc.tile_pool(name="ps", bufs=1, space="PSUM") as ps,
            tc.tile_pool(name="ps_big", bufs=1, space="PSUM") as psb,
        ):
            # ---- inputs to SBUF (x first — it gates compute; two HWDGE rings) ----
            # CZ = [x^T (0:64) | zs (64:80) | zc (80:96) | ones (96)]
            cz = sb.tile([NFEAT, S], dt.bfloat16)
            nc.sync.dma_start(out=cz[0:D, :], in_=xT[:])

            c_pk = cpool.tile([128, 361], dt.bfloat16)
            nc.scalar.dma_start(out=c_pk[:], in_=pk[:])
            c_wsc = c_pk[0:D, 0:64]
            c_o1 = c_pk[0:128, 64:72]
            c_e2q = c_pk[0:8, 72:200]
            c_o2 = c_pk[0:128, 200:232]
            c_g = c_pk[0:NFEAT, 232:360]

            # pre-seed the output with x (residual) straight in DRAM; the
            # per-range output DMAs accumulate the gated term on top.
            xcopy_inst = nc.gpsimd.dma_start(out=out_d[:], in_=xn[:])

            # trig table lives on partitions 64..95 to lane-align with CZ
            c_trig = cpool.tile([96, S], dt.bfloat16)
            nc.sync.dma_start(out=c_trig[64:96, :], in_=trig[:])

            # exp bias column (fp32 for the activation bias operand)
            c_bsc = cpool.tile([128, 1], dt.float32)
            nc.vector.tensor_copy(c_bsc[:], c_pk[:, 360:361])

            nc.vector.memset(cz[NFEAT - 1:NFEAT, :], 1.0)

            # warm up the activation table set (exp/tanh share one set)
            warm = cpool.tile([1, 2], dt.float32)
            nc.vector.memset(warm[:], 0.0)
            nc.scalar.activation(warm[:], warm[:], AF.Exp)

            for c in range(nchunks):
                lo = c * CW                      # stacked column offset
                # token ranges covered by this chunk (one per half)
                tok_los = (lo, HALF + lo)

                # -- scores: S2[half*64+hf, col] = sum_d x^T[d, tok] Wsc[d, hf]
                s2 = ps2.tile([128, CW], dt.float32, tag="s2")
                for h in range(2):
                    t0 = tok_los[h]
                    for n0 in range(0, CW, 512):
                        nn = min(512, CW - n0)
                        nc.tensor.matmul(
                            s2[h * 64:(h + 1) * 64, n0:n0 + nn],
                            c_wsc,
                            cz[0:D, t0 + n0:t0 + n0 + nn],
                            tile_position=(0, h * 64),
                        )

                # -- exp (bias adds the constant alpha-offset term)
                e1 = wk.tile([128, CW], dt.bfloat16, tag="e1")
                nc.scalar.activation(e1[:], s2[:], AF.Exp, bias=c_bsc[:])

                # -- denominators: den[(half,h), col] = sum_f e1
                den = ps.tile([8, CW], dt.float32, tag="den")
                for n0 in range(0, CW, 512):
                    nn = min(512, CW - n0)
                    nc.tensor.matmul(
                        den[:, n0:n0 + nn], c_o1, e1[:, n0:n0 + nn],
                        tile_position=(0, 0),
                    )

                # -- reciprocal (fast Newton approx, ~18 bits) + bf16 cast
                rec = wk.tile([8, CW], dt.float32, tag="rec")
                nc.vector.reciprocal_approx_fast(rec[:], den[:])
                recb = wk.tile([8, CW], dt.bfloat16, tag="recb")
                nc.vector.tensor_copy(recb[:], rec[:])

                # -- broadcast 1/den back to all 128 rows (x0.25 head-mean)
                rb = ps.tile([128, CW], dt.float32, tag="rb")
                for n0 in range(0, CW, 512):
                    nn = min(512, CW - n0)
                    nc.tensor.matmul(
                        rb[:, n0:n0 + nn], c_e2q, recb[:, n0:n0 + nn],
                        tile_position=(0, 0),
                    )

                # -- attn/4 = e1 * rb
                at = wk.tile([128, CW], dt.bfloat16, tag="at")
                nc.vector.tensor_mul(at[:], e1[:], rb[:])

                # -- aw rows (duplicated for sin/cos) on partitions 64..95
                aw = psb.tile([96, 2 * CW], dt.float32, tag="aw")
                for h in range(2):
                    for n0 in range(0, CW, 512):
                        nn = min(512, CW - n0)
                        nc.tensor.matmul(
                            aw[64:96, h * CW + n0:h * CW + n0 + nn],
                            c_o2[h * 64:(h + 1) * 64, :],
                            at[h * 64:(h + 1) * 64, n0:n0 + nn],
                            tile_position=(h * 64, 64),
                        )

                # -- z features into CZ rows 64..96 (aw * sin/cos table)
                for h in range(2):
                    t0 = tok_los[h]
                    nc.vector.tensor_mul(
                        cz[64:96, t0:t0 + CW],
                        aw[64:96, h * CW:(h + 1) * CW],
                        c_trig[64:96, t0:t0 + CW],
                    )

                # -- MLP: per 128-token tile, pre = CZ_tile^T @ G  (nat layout)
                pre = psb.tile([128, (KT // nchunks) * 128], dt.float32, tag="pre")
                kts = [t0 // 128 + i for t0 in tok_los for i in range(CW // 128)]
                for j, k in enumerate(kts):
                    nc.tensor.matmul(
                        pre[:, j * 128:(j + 1) * 128],
                        cz[:, k * 128:(k + 1) * 128],
                        c_g,
                        tile_position=(0, 0),
                    )

                # -- tanh(pre/2); sigmoid(a)=0.5+0.5*tanh(a/2)
                th = wk.tile([128, (KT // nchunks) * 128], dt.bfloat16, tag="th")
                nc.scalar.activation(th[:], pre[:], AF.Tanh, scale=0.5)

                nj = len(kts)
                kpc = CW // 128  # k-tiles per token range
                th_v = th[:].rearrange("p (j o) -> p j o", j=nj)
                pre_v = pre[:].rearrange("p (j o) -> p j o", j=nj)

                # -- w = (1+tanh_p) * pre_p   [silu*2]
                wt = wk.tile([128, nj * 64], dt.bfloat16, tag="wt")
                wt_v = wt[:].rearrange("p (j o) -> p j o", j=nj)
                nc.vector.scalar_tensor_tensor(
                    wt_v, th_v[:, :, 64:128], 1.0, pre_v[:, :, 64:128],
                    ALU.add, ALU.mult,
                )
                # -- gated*4 = (1+tanh_g) * w
                gt = wk.tile([128, nj * 64], dt.bfloat16, tag="gt")
                gt_v = gt[:].rearrange("p (j o) -> p j o", j=nj)
                nc.vector.scalar_tensor_tensor(
                    gt_v, th_v[:, :, 0:64], 1.0, wt_v, ALU.add, ALU.mult,
                )
                # -- gated = gt*0.25 in fp32, then ACCUMULATE onto out (=x)
                out_t = wk.tile([128, nj * 64], dt.float32, tag="outc")
                nc.vector.tensor_scalar_mul(out_t[:], gt[:], 0.25)
                out_v = out_d.rearrange("(k p) d -> p k d", p=128)
                for h in range(2):
                    k0 = tok_los[h] // 128
                    acc = nc.gpsimd.dma_start(
                        out=out_v[:, k0:k0 + kpc, :],
                        in_=out_t[:, h * kpc * 64:(h + 1) * kpc * 64],
                        accum_op=ALU.add,
                    )
                    add_dep_helper(acc.ins, xcopy_inst.ins, sync=True,
                                   reason="accumulate after x pre-seed")

    nc.compile()
    return nc


def _fold_params(inputs):
    """Host-side constant folding (float64).  Returns per-core arrays."""
    f = (np.asarray(inputs["freq_matrix"], np.float64)
         * np.asarray(inputs["freq_scale"], np.float64))
    g = f[0]
    gc = 0.5 * (g.max() + g.min())
    gsh = g - gc

    Wq = np.asarray(inputs["Wq"], np.float64)
    bq = np.asarray(inputs["bq"], np.float64)
    Wk1 = np.asarray(inputs["Wk1"], np.float64)
    bk1 = np.asarray(inputs["bk1"], np.float64)
    Wqi = np.asarray(inputs["Wqi"], np.float64)
    bqi = np.asarray(inputs["bqi"], np.float64)
    Wki = np.asarray(inputs["Wki"], np.float64)
    bki = np.asarray(inputs["bki"], np.float64)
    ph = np.asarray(inputs["phase"], np.float64)

    u = Wki @ Wk1[:, 0]
    Wqq = Wqi @ Wq
    bqq = Wqi @ bq + bqi
    u_h = u.reshape(H, HD)
    M_alpha = np.einsum("he,hed->hd", u_h, Wqq.reshape(H, HD, D)) / np.sqrt(HD)
    c_alpha = np.einsum("he,he->h", u_h, bqq.reshape(H, HD)) / np.sqrt(HD)

    W_score = np.einsum("hd,f->dhf", M_alpha, gsh).reshape(D, HF)
    b_score = np.einsum("h,f->hf", c_alpha, gsh).reshape(HF)
    b_score2 = np.concatenate([b_score, b_score]).reshape(128, 1)

    t = np.linspace(0.0, 1.0, S)
    theta = TWO_PI * t[:, None] * g[None, :]
    trig = np.concatenate([np.sin(theta).T, np.cos(theta).T], 0)  # [2F, S]

    cph, sph = np.cos(ph), np.sin(ph)

    def fold_mlp(W):
        W = np.asarray(W, np.float64)
        Wx = W[:, :D]
        Wf = W[:, D:].reshape(O, D, 2 * F)
        Ws, Wc = Wf[:, :, :F], Wf[:, :, F:]
        Us = np.einsum("df,odf->fo", cph, Ws) - np.einsum("df,odf->fo", sph, Wc)
        Uc = np.einsum("df,odf->fo", sph, Ws) + np.einsum("df,odf->fo", cph, Wc)
        return Wx, Us, Uc

    Wgx, Ugs, Ugc = fold_mlp(inputs["Wg"])
    Wpx, Ups, Upc = fold_mlp(inputs["Wp"])
    bg = np.asarray(inputs["bg"], np.float64)
    bp = np.asarray(inputs["bp"], np.float64)

    G = np.zeros((NFEAT, 128))
    G[0:D, 0:64] = Wgx.T
    G[D:D + F, 0:64] = Ugs
    G[D + F:D + 2 * F, 0:64] = Ugc
    G[NFEAT - 1, 0:64] = bg
    G[0:D, 64:128] = Wpx.T
    G[D:D + F, 64:128] = Ups
    G[D + F:D + 2 * F, 64:128] = Upc
    G[NFEAT - 1, 64:128] = bp

    # indicator matrices for the softmax plumbing
    p = np.arange(128)
    O1 = (p[:, None] // 16 == np.arange(8)[None, :]).astype(np.float64)
    E2q = 0.25 * (np.arange(8)[:, None] == p[None, :] // 16).astype(np.float64)
    O2 = ((p[:, None] % 16) == (np.arange(32)[None, :] % 16)).astype(np.float64)

    # pack all bf16 params into one [128, 361] array (see _build_program)
    pk = np.zeros((128, 361))
    pk[0:D, 0:64] = W_score
    pk[0:128, 64:72] = O1
    pk[0:8, 72:200] = E2q
    pk[0:128, 200:232] = O2
    pk[0:NFEAT, 232:360] = G
    pk[:, 360] = b_score2[:, 0]

    return dict(
        trig=trig.astype(BF16),
        pk=pk.astype(BF16),
    ), gsh, M_alpha, c_alpha


def _numpy_fallback(inputs):
    """Exact collapsed computation in numpy (general freq rows)."""
    x = np.asarray(inputs["x"], np.float64)
    f = (np.asarray(inputs["freq_matrix"], np.float64)
         * np.asarray(inputs["freq_scale"], np.float64))
    Wq = np.asarray(inputs["Wq"], np.float64); bq = np.asarray(inputs["bq"], np.float64)
    Wk1 = np.asarray(inputs["Wk1"], np.float64); bk1 = np.asarray(inputs["bk1"], np.float64)
    Wqi = np.asarray(inputs["Wqi"], np.float64); bqi = np.asarray(inputs["bqi"], np.float64)
    Wki = np.asarray(inputs["Wki"], np.float64); bki = np.asarray(inputs["bki"], np.float64)
    Wg = np.asarray(inputs["Wg"], np.float64); bg = np.asarray(inputs["bg"], np.float64)
    Wp = np.asarray(inputs["Wp"], np.float64); bp = np.asarray(inputs["bp"], np.float64)
    ph = np.asarray(inputs["phase"], np.float64)

    u = Wki @ Wk1[:, 0]
    v = Wki @ bk1 + bki
    q = (x @ Wq.T + bq) @ Wqi.T + bqi                      # [B,S,A]
    qh = q.reshape(B, S, H, HD)
    alpha = np.einsum("bshe,he->bsh", qh, u.reshape(H, HD)) / np.sqrt(HD)
    beta = np.einsum("bshe,he->bsh", qh, v.reshape(H, HD)) / np.sqrt(HD)
    sc = alpha[..., None, :, None] * f[None, None, :, None, :] \
        + beta[..., None, :, None]                         # [B,S,D,H,F]
    sc -= sc.max(-1, keepdims=True)
    e = np.exp(sc)
    attn = e / e.sum(-1, keepdims=True)
    aw = attn.mean(-2)                                     # [B,S,D,F]
    t = np.linspace(0.0, 1.0, S)
    sig = TWO_PI * t[None, :, None, None] * f[None, None] + ph[None, None]
    ffs = np.sin(sig) * aw
    ffc = np.cos(sig) * aw
    ff = np.concatenate([ffs, ffc], axis=-1).reshape(B, S, D * 2 * F)
    ci = np.concatenate([x, ff], axis=-1)
    gate = 1.0 / (1.0 + np.exp(-(ci @ Wg.T + bg)))
    pp = ci @ Wp.T + bp
    silu = pp / (1.0 + np.exp(-pp))
    return (x + gate * silu).astype(np.float32)


def kernel(**inputs) -> np.ndarray:
    x = np.asarray(inputs["x"], np.float32)

    f = (np.asarray(inputs["freq_matrix"], np.float64)
         * np.asarray(inputs["freq_scale"], np.float64))
    if not np.all(f == f[0:1]):
        return _numpy_fallback(inputs)

    params, gsh, M_alpha, c_alpha = _fold_params(inputs)

    # exp-overflow guard (score = alpha*(g-gc); needs |score| < ~85)
    xmaxn = np.linalg.norm(x.reshape(-1, D), axis=1).max()
    amax = np.linalg.norm(M_alpha, axis=1).max() * xmaxn + np.abs(c_alpha).max()
    if amax * np.abs(gsh).max() > 85.0:
        return _numpy_fallback(inputs)

    key = "prog"
    if key not in _CACHE:
        _CACHE[key] = _build_program()
    nc = _CACHE[key]

    from concourse.bass_utils import run_bass_kernel_spmd

    in_maps = []
    for b in range(N_CORES):
        m = dict(params)
        m["xT"] = np.ascontiguousarray(x[b].T).astype(BF16)
        m["xn"] = np.ascontiguousarray(x[b])
        in_maps.append(m)

    res = run_bass_kernel_spmd(nc, in_maps, core_ids=list(range(N_CORES)))
    out = np.stack([res.results[b]["out"] for b in range(N_CORES)], axis=0)
    return out.astype(np.float32)


if __name__ == "__main__":
    import reference
    ins = {k: np.asarray(v) for k, v in reference.setup_inputs().items()}
    got = kernel(**ins)
    import jax.numpy as jnp
    exp = np.asarray(reference.reference(**{k: jnp.asarray(v) for k, v in ins.items()}))
    err = np.linalg.norm(got - exp) / np.linalg.norm(exp)
    print("rel err:", err)


# revision 28
# speedup vs baseline: 1.3517x; 1.1063x over previous
"""Trainium2 Bass kernel for nn_AdaptiveFourierFeatures.

Strategy
--------
The reference module computes, per batch b and token s:

    q[s,h,:]   depends on x[s] through two linear layers
    k[d,f,h,:] = f[d,f]*u[h,:] + v[h,:]         (keys are AFFINE in f[d,f]
                                                  because key_proj is Linear(1,A))
    scores[s,d,h,f] = q.k/sqrt(HD) = alpha[s,h]*f[d,f] + beta[s,h]

With the given inputs, freq_matrix*freq_scale has IDENTICAL rows
(f[d,:] == g[:] for all d), so softmax over f is d-independent and beta
cancels inside the softmax:

    attn[s,h,f] = softmax_f(alpha[s,h] * (g[f]-gc))      (gc: shift for range)
    aw[s,f]     = mean_h attn[s,h,f]

The fourier features contract with the MLP weights analytically using
sin(theta+phi) = sin*cos + cos*sin, folding phase and the D dimension into
small [F,O] matrices on the host.  The device pipeline per token is then:

    x(64) -> alpha-scores(64=H*F) -> softmax -> aw features z(32)
          -> [x|z|1](97) @ G(97x128) -> sigmoid*silu gate -> residual

Sharding: data-parallel over batch B=8, one batch element per NeuronCore.
All folded parameters are tiny and replicated.

kernel(**inputs) takes the FULL inputs and returns the FULL [B,S,D] output.
"""

import numpy as np
import ml_dtypes

# ---- problem constants (hardcoded; kernel.py must be self-contained) ----
B, S, D, F, A, H, O = 8, 2048, 64, 16, 32, 4, 64
HD = A // H
TWO_PI = 2.0 * np.pi
N_CORES = 8
HF = H * F            # 64 score columns per token
NFEAT = D + 2 * F + 1  # 97 = x | z_sin | z_cos | ones
HALF = S // 2          # stacked-half layout: 1024 tokens per half

BF16 = ml_dtypes.bfloat16

_CACHE = {}


def _build_program(nchunks: int = 2):
    """Build the 8-core SPMD bass program (per-core shapes)."""
    import concourse.bass as bass
    import concourse.bacc as bacc
    import concourse.tile as tile
    from concourse import mybir
    from bass_rust import add_dep_helper

    dt = mybir.dt
    AF = mybir.ActivationFunctionType
    ALU = mybir.AluOpType

    nc = bacc.Bacc("TRN2", target_bir_lowering=False, debug=False,
                   num_devices=N_CORES)

    # ---- per-core DRAM parameters ----
    # all bf16 params packed into one [128, 361] array:
    #   wsc [64,64] @cols 0:64, o1 [128,8] @64:72, e2q [8,128] @72:200,
    #   o2 [128,32] @200:232, G [97,128] @232:360, b_score [128,1] @360
    xT = nc.dram_tensor("xT", [D, S], dt.bfloat16, kind="ExternalInput").ap()
    xn = nc.dram_tensor("xn", [S, D], dt.float32, kind="ExternalInput").ap()
    trig = nc.dram_tensor("trig", [2 * F, S], dt.bfloat16, kind="ExternalInput").ap()
    pk = nc.dram_tensor("pk", [128, 361], dt.bfloat16, kind="ExternalInput").ap()
    out_d = nc.dram_tensor("out", [S, D], dt.float32, kind="ExternalOutput").ap()

    KT = S // 128                # 16 token tiles of 128
    CW = HALF // nchunks         # stacked-column chunk width

    with tile.TileContext(nc) as tc:
        with (
            tc.tile_pool(name="const", bufs=1) as cpool,
            tc.tile_pool(name="sb", bufs=1) as sb,
            tc.tile_pool(name="work", bufs=2) as wk,
            tc.tile_pool(name="ps2", bufs=2, space="PSUM") as ps2,
            tc.tile_pool(name="ps", bufs=1, space="PSUM") as ps,
            tc.tile_pool(name="ps_big", bufs=1, space="PSUM") as psb,
        ):
            # ---- inputs to SBUF (x first — it gates compute; two HWDGE rings) ----
            # CZ = [x^T (0:64) | zs (64:80) | zc (80:96) | ones (96)]
            cz = sb.tile([NFEAT, S], dt.bfloat16)
            nc.sync.dma_start(out=cz[0:D, :], in_=xT[:])

            c_pk = cpool.tile([128, 361], dt.bfloat16)
            nc.scalar.dma_start(out=c_pk[:], in_=pk[:])
            c_wsc = c_pk[0:D, 0:64]
            c_o1 = c_pk[0:128, 64:72]
            c_e2q = c_pk[0:8, 72:200]
            c_o2 = c_pk[0:128, 200:232]
            c_g = c_pk[0:NFEAT, 232:360]

            # natural-layout x for the residual: [128, (k,64)]
            xn_t = sb.tile([128, KT * D], dt.float32)
            nc.scalar.dma_start(
                out=xn_t[:],
                in_=xn.rearrange("(k p) d -> p k d", p=128),
            )

            # trig table lives on partitions 64..95 to lane-align with CZ
            c_trig = cpool.tile([96, S], dt.bfloat16)
            nc.sync.dma_start(out=c_trig[64:96, :], in_=trig[:])

            # exp bias column (fp32 for the activation bias operand)
            c_bsc = cpool.tile([128, 1], dt.float32)
            nc.vector.tensor_copy(c_bsc[:], c_pk[:, 360:361])

            nc.vector.memset(cz[NFEAT - 1:NFEAT, :], 1.0)

            # warm up the activation table set (exp/tanh share one set)
            warm = cpool.tile([1, 2], dt.float32)
            nc.vector.memset(warm[:], 0.0)
            nc.scalar.activation(warm[:], warm[:], AF.Exp)

            for c in range(nchunks):
                lo = c * CW                      # stacked column offset
                # token ranges covered by this chunk (one per half)
                tok_los = (lo, HALF + lo)

                # -- scores: S2[half*64+hf, col] = sum_d x^T[d, tok] Wsc[d, hf]
                s2 = ps2.tile([128, CW], dt.float32, tag="s2")
                for h in range(2):
                    t0 = tok_los[h]
                    for n0 in range(0, CW, 512):
                        nn = min(512, CW - n0)
                        nc.tensor.matmul(
                            s2[h * 64:(h + 1) * 64, n0:n0 + nn],
                            c_wsc,
                            cz[0:D, t0 + n0:t0 + n0 + nn],
                            tile_position=(0, h * 64),
                        )

                # -- exp (bias adds the constant alpha-offset term)
                e1 = wk.tile([128, CW], dt.bfloat16, tag="e1")
                nc.scalar.activation(e1[:], s2[:], AF.Exp, bias=c_bsc[:])

                # -- denominators: den[(half,h), col] = sum_f e1
                den = ps.tile([8, CW], dt.float32, tag="den")
                for n0 in range(0, CW, 512):
                    nn = min(512, CW - n0)
                    nc.tensor.matmul(
                        den[:, n0:n0 + nn], c_o1, e1[:, n0:n0 + nn],
                        tile_position=(0, 0),
                    )

                # -- reciprocal (fast Newton approx, ~18 bits) + bf16 cast
                rec = wk.tile([8, CW], dt.float32, tag="rec")
                nc.vector.reciprocal_approx_fast(rec[:], den[:])
                recb = wk.tile([8, CW], dt.bfloat16, tag="recb")
                nc.vector.tensor_copy(recb[:], rec[:])

                # -- broadcast 1/den back to all 128 rows (x0.25 head-mean)
                rb = ps.tile([128, CW], dt.float32, tag="rb")
                for n0 in range(0, CW, 512):
                    nn = min(512, CW - n0)
                    nc.tensor.matmul(
                        rb[:, n0:n0 + nn], c_e2q, recb[:, n0:n0 + nn],
                        tile_position=(0, 0),
                    )

                # -- attn/4 = e1 * rb
                at = wk.tile([128, CW], dt.bfloat16, tag="at")
                nc.vector.tensor_mul(at[:], e1[:], rb[:])

                # -- aw rows (duplicated for sin/cos) on partitions 64..95
                aw = psb.tile([96, 2 * CW], dt.float32, tag="aw")
                for h in range(2):
                    for n0 in range(0, CW, 512):
                        nn = min(512, CW - n0)
                        nc.tensor.matmul(
                            aw[64:96, h * CW + n0:h * CW + n0 + nn],
                            c_o2[h * 64:(h + 1) * 64, :],
                            at[h * 64:(h + 1) * 64, n0:n0 + nn],
                            tile_position=(h * 64, 64),
                        )

                # -- z features into CZ rows 64..96 (aw * sin/cos table);
                # one TT over both halves via multi-run column APs
                cz_v = cz[64:96, :].rearrange("p (h c) -> p h c", h=2)[:, :, lo:lo + CW]
                trig_v = c_trig[64:96, :].rearrange("p (h c) -> p h c", h=2)[:, :, lo:lo + CW]
                aw_v = aw[64:96, :].rearrange("p (h c) -> p h c", h=2)
                nc.vector.tensor_mul(cz_v, aw_v, trig_v)

                # -- MLP: per 128-token tile, pre = CZ_tile^T @ G  (nat layout)
                pre = psb.tile([128, (KT // nchunks) * 128], dt.float32, tag="pre")
                kts = [t0 // 128 + i for t0 in tok_los for i in range(CW // 128)]
                for j, k in enumerate(kts):
                    nc.tensor.matmul(
                        pre[:, j * 128:(j + 1) * 128],
                        cz[:, k * 128:(k + 1) * 128],
                        c_g,
                        tile_position=(0, 0),
                    )

                # -- tanh(pre/2); sigmoid(a)=0.5+0.5*tanh(a/2)
                th = wk.tile([128, (KT // nchunks) * 128], dt.bfloat16, tag="th")
                nc.scalar.activation(th[:], pre[:], AF.Tanh, scale=0.5)

                nj = len(kts)
                kpc = CW // 128  # k-tiles per token range
                th_v = th[:].rearrange("p (j o) -> p j o", j=nj)
                pre_v = pre[:].rearrange("p (j o) -> p j o", j=nj)

                # -- w = (1+tanh_p) * pre_p   [silu*2]
                wt = wk.tile([128, nj * 64], dt.bfloat16, tag="wt")
                wt_v = wt[:].rearrange("p (j o) -> p j o", j=nj)
                nc.vector.scalar_tensor_tensor(
                    wt_v, th_v[:, :, 64:128], 1.0, pre_v[:, :, 64:128],
                    ALU.add, ALU.mult,
                )
                # -- gated*4 = (1+tanh_g) * w
                gt = wk.tile([128, nj * 64], dt.bfloat16, tag="gt")
                gt_v = gt[:].rearrange("p (j o) -> p j o", j=nj)
                nc.vector.scalar_tensor_tensor(
                    gt_v, th_v[:, :, 0:64], 1.0, wt_v, ALU.add, ALU.mult,
                )
                # -- out = gated*0.25 + x ; DMA out per token range (HWDGE)
                out_t = wk.tile([128, nj * 64], dt.float32, tag="outc")
                out_v = out_d.rearrange("(k p) d -> p k d", p=128)
                for h in range(2):
                    k0 = tok_los[h] // 128
                    sl = slice(h * kpc * 64, (h + 1) * kpc * 64)
                    nc.vector.scalar_tensor_tensor(
                        out_t[:, sl], gt[:, sl], 0.25,
                        xn_t[:, k0 * 64:(k0 + kpc) * 64],
                        ALU.mult, ALU.add,
                    )
                    eng = nc.sync if h == 0 else nc.scalar
                    eng.dma_start(out=out_v[:, k0:k0 + kpc, :], in_=out_t[:, sl])

    nc.compile()
    return nc


def _fold_params(inputs):
    """Host-side constant folding (float64).  Returns per-core arrays."""
    f = (np.asarray(inputs["freq_matrix"], np.float64)
         * np.asarray(inputs["freq_scale"], np.float64))
    g = f[0]
    gc = 0.5 * (g.max() + g.min())
    gsh = g - gc

    Wq = np.asarray(inputs["Wq"], np.float64)
    bq = np.asarray(inputs["bq"], np.float64)
    Wk1 = np.asarray(inputs["Wk1"], np.float64)
    bk1 = np.asarray(inputs["bk1"], np.float64)
    Wqi = np.asarray(inputs["Wqi"], np.float64)
    bqi = np.asarray(inputs["bqi"], np.float64)
    Wki = np.asarray(inputs["Wki"], np.float64)
    bki = np.asarray(inputs["bki"], np.float64)
    ph = np.asarray(inputs["phase"], np.float64)

    u = Wki @ Wk1[:, 0]
    Wqq = Wqi @ Wq
    bqq = Wqi @ bq + bqi
    u_h = u.reshape(H, HD)
    M_alpha = np.einsum("he,hed->hd", u_h, Wqq.reshape(H, HD, D)) / np.sqrt(HD)
    c_alpha = np.einsum("he,he->h", u_h, bqq.reshape(H, HD)) / np.sqrt(HD)

    W_score = np.einsum("hd,f->dhf", M_alpha, gsh).reshape(D, HF)
    b_score = np.einsum("h,f->hf", c_alpha, gsh).reshape(HF)
    b_score2 = np.concatenate([b_score, b_score]).reshape(128, 1)

    t = np.linspace(0.0, 1.0, S)
    theta = TWO_PI * t[:, None] * g[None, :]
    trig = np.concatenate([np.sin(theta).T, np.cos(theta).T], 0)  # [2F, S]

    cph, sph = np.cos(ph), np.sin(ph)

    def fold_mlp(W):
        W = np.asarray(W, np.float64)
        Wx = W[:, :D]
        Wf = W[:, D:].reshape(O, D, 2 * F)
        Ws, Wc = Wf[:, :, :F], Wf[:, :, F:]
        Us = np.einsum("df,odf->fo", cph, Ws) - np.einsum("df,odf->fo", sph, Wc)
        Uc = np.einsum("df,odf->fo", sph, Ws) + np.einsum("df,odf->fo", cph, Wc)
        return Wx, Us, Uc

    Wgx, Ugs, Ugc = fold_mlp(inputs["Wg"])
    Wpx, Ups, Upc = fold_mlp(inputs["Wp"])
    bg = np.asarray(inputs["bg"], np.float64)
    bp = np.asarray(inputs["bp"], np.float64)

    G = np.zeros((NFEAT, 128))
    G[0:D, 0:64] = Wgx.T
    G[D:D + F, 0:64] = Ugs
    G[D + F:D + 2 * F, 0:64] = Ugc
    G[NFEAT - 1, 0:64] = bg
    G[0:D, 64:128] = Wpx.T
    G[D:D + F, 64:128] = Ups
    G[D + F:D + 2 * F, 64:128] = Upc
    G[NFEAT - 1, 64:128] = bp

    # indicator matrices for the softmax plumbing
    p = np.arange(128)
    O1 = (p[:, None] // 16 == np.arange(8)[None, :]).astype(np.float64)
    E2q = 0.25 * (np.arange(8)[:, None] == p[None, :] // 16).astype(np.float64)
    O2 = ((p[:, None] % 16) == (np.arange(32)[None, :] % 16)).astype(np.float64)

    # pack all bf16 params into one [128, 361] array (see _build_program)
    pk = np.zeros((128, 361))
    pk[0:D, 0:64] = W_score
    pk[0:128, 64:72] = O1
    pk[0:8, 72:200] = E2q
    pk[0:128, 200:232] = O2
    pk[0:NFEAT, 232:360] = G
    pk[:, 360] = b_score2[:, 0]

    return dict(
        trig=trig.astype(BF16),
        pk=pk.astype(BF16),
    ), gsh, M_alpha, c_alpha


def _numpy_fallback(inputs):
    """Exact collapsed computation in numpy (general freq rows)."""
    x = np.asarray(inputs["x"], np.float64)
    f = (np.asarray(inputs["freq_matrix"], np.float64)
         * np.asarray(inputs["freq_scale"], np.float64))
    Wq = np.asarray(inputs["Wq"], np.float64); bq = np.asarray(inputs["bq"], np.float64)
    Wk1 = np.asarray(inputs["Wk1"], np.float64); bk1 = np.asarray(inputs["bk1"], np.float64)
    Wqi = np.asarray(inputs["Wqi"], np.float64); bqi = np.asarray(inputs["bqi"], np.float64)
    Wki = np.asarray(inputs["Wki"], np.float64); bki = np.asarray(inputs["bki"], np.float64)
    Wg = np.asarray(inputs["Wg"], np.float64); bg = np.asarray(inputs["bg"], np.float64)
    Wp = np.asarray(inputs["Wp"], np.float64); bp = np.asarray(inputs["bp"], np.float64)
    ph = np.asarray(inputs["phase"], np.float64)

    u = Wki @ Wk1[:, 0]
    v = Wki @ bk1 + bki
    q = (x @ Wq.T + bq) @ Wqi.T + bqi                      # [B,S,A]
    qh = q.reshape(B, S, H, HD)
    alpha = np.einsum("bshe,he->bsh", qh, u.reshape(H, HD)) / np.sqrt(HD)
    beta = np.einsum("bshe,he->bsh", qh, v.reshape(H, HD)) / np.sqrt(HD)
    sc = alpha[..., None, :, None] * f[None, None, :, None, :] \
        + beta[..., None, :, None]                         # [B,S,D,H,F]
    sc -= sc.max(-1, keepdims=True)
    e = np.exp(sc)
    attn = e / e.sum(-1, keepdims=True)
    aw = attn.mean(-2)                                     # [B,S,D,F]
    t = np.linspace(0.0, 1.0, S)
    sig = TWO_PI * t[None, :, None, None] * f[None, None] + ph[None, None]
    ffs = np.sin(sig) * aw
    ffc = np.cos(sig) * aw
    ff = np.concatenate([ffs, ffc], axis=-1).reshape(B, S, D * 2 * F)
    ci = np.concatenate([x, ff], axis=-1)
    gate = 1.0 / (1.0 + np.exp(-(ci @ Wg.T + bg)))
    pp = ci @ Wp.T + bp
    silu = pp / (1.0 + np.exp(-pp))
    return (x + gate * silu).astype(np.float32)


def kernel(**inputs) -> np.ndarray:
    x = np.asarray(inputs["x"], np.float32)

    f = (np.asarray(inputs["freq_matrix"], np.float64)
         * np.asarray(inputs["freq_scale"], np.float64))
    if not np.all(f == f[0:1]):
        return _numpy_fallback(inputs)

    params, gsh, M_alpha, c_alpha = _fold_params(inputs)

    # exp-overflow guard (score = alpha*(g-gc); needs |score| < ~85)
    xmaxn = np.linalg.norm(x.reshape(-1, D), axis=1).max()
    amax = np.linalg.norm(M_alpha, axis=1).max() * xmaxn + np.abs(c_alpha).max()
    if amax * np.abs(gsh).max() > 85.0:
        return _numpy_fallback(inputs)

    key = "prog"
    if key not in _CACHE:
        _CACHE[key] = _build_program()
    nc = _CACHE[key]

    from concourse.bass_utils import run_bass_kernel_spmd

    in_maps = []
    for b in range(N_CORES):
        m = dict(params)
        m["xT"] = np.ascontiguousarray(x[b].T).astype(BF16)
        m["xn"] = np.ascontiguousarray(x[b])
        in_maps.append(m)

    res = run_bass_kernel_spmd(nc, in_maps, core_ids=list(range(N_CORES)))
    out = np.stack([res.results[b]["out"] for b in range(N_CORES)], axis=0)
    return out.astype(np.float32)


if __name__ == "__main__":
    import reference
    ins = {k: np.asarray(v) for k, v in reference.setup_inputs().items()}
    got = kernel(**ins)
    import jax.numpy as jnp
    exp = np.asarray(reference.reference(**{k: jnp.asarray(v) for k, v in ins.items()}))
    err = np.linalg.norm(got - exp) / np.linalg.norm(exp)
    print("rel err:", err)
